# revision 2
# baseline (speedup 1.0000x reference)
"""Self-contained GCN kernel for trn2 (8 NeuronCores). kernel(**inputs) -> [N,1] fp32."""
import sys
sys.path.insert(0, "/opt/trn_rl_repo")
import numpy as np

P = 128
CHUNK = 128
MAXIDX = 2048
N = 100000
NCORES = 8
LAYER_DIMS = [(3, 128), (128, 128), (128, 64), (64, 64), (64, 1)]

_cache = {}


def kernel(x, edge_index, W1, b1, W2, b2, W3, b3, W4, b4, W5, b5):
    import ml_dtypes
    from concourse.bass_utils import run_bass_kernel_spmd
    from gcn_builder import prepare, build

    x = np.asarray(x, np.float32)
    key = "k"
    if key not in _cache:
        cfg, per_core, common, dis = prepare(N, NCORES, np.asarray(edge_index), x)
        nc = build(cfg, LAYER_DIMS)
        _cache[key] = (cfg, per_core, common, nc)
    cfg, per_core, common, nc = _cache[key]

    bf16 = ml_dtypes.bfloat16
    Ws = [np.asarray(w, np.float32).astype(bf16) for w in (W1, W2, W3, W4, W5)]
    bs = [np.asarray(b, np.float32) for b in (b1, b2, b3, b4, b5)]
    in_maps = []
    for c in range(NCORES):
        m = dict(per_core[c])
        m.update(common)
        for l in range(1, 6):
            m[f"W{l}"] = Ws[l - 1]
            bt = np.zeros((P, 1), np.float32)
            bt[:bs[l - 1].size, 0] = bs[l - 1]
            m[f"b{l}"] = bt
        in_maps.append(m)

    res = run_bass_kernel_spmd(nc, in_maps, list(range(NCORES)))
    out = np.concatenate([res.results[c]["out"] for c in range(NCORES)], axis=0)
    return out.astype(np.float32)
